# revision 4
# baseline (speedup 1.0000x reference)
"""Cross-attention kernel for Trainium2, 8 NeuronCores, data-parallel over batch.

Reference computation (per batch element b):
    Q = target[b] @ Wq.T + bq            [T, D]
    K = source[b] @ Wk.T + bk            [T, D]
    V = source[b] @ Wv.T + bv            [T, D]
    S = Q @ K.T / sqrt(D)                [T, T]
    A = softmax(S, axis=-1)
    O = A @ V                            [T, D]
    Y = O @ Wo.T + bo                    [T, D]

Sharding: B=8 batch elements -> 8 cores, one each; weights replicated.

Device-side layout strategy (everything "transposed" so each matmul gets
natural operands and softmax reductions happen via PE ones-matmuls):
    host feeds   tgtT = target[b].T [D,T],  srcT = source[b].T [D,T],
                 wqT/wkT/wvT = W.T [D,D],   woT = Wo.T [D,D]
    device:      KT[e,s] = wkT.T @ srcT + bk        (phase A1)
                 V[s,e]  = srcT.T-tiles @ wvT + bv  (phase A2)
                 QT[e,t] = wqT.T @ tgtT + bq -> DRAM spill (phase B1)
                 per t-chunk:
                   E[s,t]    = exp((KT.T @ QT)/32)     unnormalized, no max-sub
                   sumexp[t] = ones.T @ E              (M=1 matmul)
                   rbc[:,t]  = ones-col outer recip    (K=1 matmul broadcast)
                   OT[e,t]   = V-tiles.T @ E           (unnormalized A@V, transposed)
                   YT[o,t]   = (woT.T @ OT) * rbc + bo -> DRAM out
    host: out[b] = YT.T

Scores stay in a safe fp32 range (|S|/32 < ~6) so softmax without
max-subtraction is numerically fine.  All matmul operands are float32r
(full-rate PE mode at moving-dim >= 256); accumulation is fp32 in PSUM.
"""

import numpy as np

import concourse.bass as bass
import concourse.mybir as mybir
import concourse.tile as tile
from concourse import bacc
from concourse.bass_utils import run_bass_kernel_spmd

F32R = mybir.dt.float32r
F32 = mybir.dt.float32

B = 8          # batch == number of cores
D = 1024       # embed dim
T = 2048       # sequence length (both target and source)
P = 128        # partitions
NDT = D // P   # 8 contraction tiles
NET = D // P   # 8 embed tiles
NST = T // P   # 16 sequence tiles
TC = 512       # t/s-chunk width in phases A1/B1
NTC = T // TC  # 4
EC = 256       # e-chunk for the V projection
NEC = D // EC  # 4
TCB = 256      # t-chunk width in phase B2 (SBUF-bound)
NTCB = T // TCB  # 8
SCALE = 1.0 / np.sqrt(np.float32(D))

_CACHED_NC = None


def build_nc():
    nc = bacc.Bacc("TRN2", target_bir_lowering=False, debug=False)

    tgtT = nc.dram_tensor("tgtT", [D, T], F32R, kind="ExternalInput").ap()
    srcT = nc.dram_tensor("srcT", [D, T], F32R, kind="ExternalInput").ap()
    wqT = nc.dram_tensor("wqT", [D, D], F32R, kind="ExternalInput").ap()
    wkT = nc.dram_tensor("wkT", [D, D], F32R, kind="ExternalInput").ap()
    wvT = nc.dram_tensor("wvT", [D, D], F32R, kind="ExternalInput").ap()
    woT = nc.dram_tensor("woT", [D, D], F32R, kind="ExternalInput").ap()
    bq = nc.dram_tensor("bq", [D, 1], F32, kind="ExternalInput").ap()
    bk = nc.dram_tensor("bk", [D, 1], F32, kind="ExternalInput").ap()
    bvb = nc.dram_tensor("bvb", [P, D], F32, kind="ExternalInput").ap()
    bo = nc.dram_tensor("bo", [D, 1], F32, kind="ExternalInput").ap()
    ones_in = nc.dram_tensor("ones_in", [P, P], F32R, kind="ExternalInput").ap()
    yT = nc.dram_tensor("yT", [D, T], F32, kind="ExternalOutput").ap()

    # tiled DRAM views: leading dim split into (tile, partition)
    tgt_r = tgtT.rearrange("(dt p) t -> p dt t", p=P)
    src_r = srcT.rearrange("(dt p) t -> p dt t", p=P)
    wq_r = wqT.rearrange("(dt p) e -> p dt e", p=P)
    wk_r = wkT.rearrange("(dt p) e -> p dt e", p=P)
    wv_r = wvT.rearrange("(dt p) e -> p dt e", p=P)
    wo_r = woT.rearrange("(et p) o -> p et o", p=P)
    bq_r = bq.rearrange("(et p) o -> p et o", p=P)
    bk_r = bk.rearrange("(et p) o -> p et o", p=P)
    bo_r = bo.rearrange("(ot p) o -> p ot o", p=P)
    y_r = yT.rearrange("(ot p) t -> p ot t", p=P)

    with tile.TileContext(nc) as tc:
        with tc.tile_pool(name="res", bufs=1) as res, \
             tc.tile_pool(name="dsp", bufs=1, space="DRAM") as dsp:
            KT = res.tile([P, NET, T], F32R, tag="kt")     # 64 KB/partition
            V = res.tile([P, NST, D], F32R, tag="v")       # 64 KB/partition
            qts = dsp.tile([P, NET, T], F32R, tag="qts")   # QT spill in DRAM

            # ---------------- Phase A: KT and V from source ----------------
            with tc.tile_pool(name="pa", bufs=1) as pa, \
                 tc.tile_pool(name="psA", bufs=1, space="PSUM") as psA:
                bk_sb = pa.tile([P, NET, 1], F32, tag="bk")
                bvb_sb = pa.tile([P, D], F32, tag="bvb")
                nc.sync.dma_start(out=bk_sb, in_=bk_r)
                nc.sync.dma_start(out=bvb_sb, in_=bvb)

                # A1: KT[e,s] = sum_d wkT[d,e] * srcT[d,s]  (+bk)
                for sc in range(NTC):
                    src_c = pa.tile([P, NDT, TC], F32R, tag="src", bufs=2)
                    nc.sync.dma_start(
                        out=src_c, in_=src_r[:, :, sc * TC:(sc + 1) * TC])
                    for et in range(NET):
                        wk_c = pa.tile([P, NDT, P], F32R, tag="wk", bufs=3)
                        nc.sync.dma_start(
                            out=wk_c, in_=wk_r[:, :, et * P:(et + 1) * P])
                        ps = psA.tile([P, TC], F32, tag="psk", bufs=2)
                        for d in range(NDT):
                            nc.tensor.matmul(
                                ps[:], wk_c[:, d, :], src_c[:, d, :],
                                start=(d == 0), stop=(d == NDT - 1))
                        nc.vector.tensor_scalar_add(
                            KT[:, et, sc * TC:(sc + 1) * TC], ps[:],
                            bk_sb[:, et, :])

                # A2: V[s,e] = sum_d srcT[d,s] * wvT[d,e]  (+bv)
                # wvT loaded in halves to fit SBUF; srcT re-read per half
                for half in range(2):
                    HD = D // 2
                    wv_h = pa.tile([P, NDT, HD], F32R, tag="wv", bufs=1)
                    nc.sync.dma_start(
                        out=wv_h, in_=wv_r[:, :, half * HD:(half + 1) * HD])
                    for st in range(NST):
                        src_s = pa.tile([P, NDT, P], F32R, tag="src2", bufs=2)
                        nc.sync.dma_start(
                            out=src_s, in_=src_r[:, :, st * P:(st + 1) * P])
                        for ec in range(NEC // 2):
                            eo = half * HD + ec * EC   # global e offset
                            ps = psA.tile([P, EC], F32, tag="psv", bufs=2)
                            for d in range(NDT):
                                nc.tensor.matmul(
                                    ps[:], src_s[:, d, :],
                                    wv_h[:, d, ec * EC:(ec + 1) * EC],
                                    start=(d == 0), stop=(d == NDT - 1))
                            nc.vector.tensor_add(
                                V[:, st, eo:eo + EC], ps[:],
                                bvb_sb[:, eo:eo + EC])

            # ---------------- Phase B1: QT -> DRAM spill ----------------
            with tc.tile_pool(name="pb", bufs=1) as pb, \
                 tc.tile_pool(name="psB", bufs=3, space="PSUM") as psB:
                bq_sb = pb.tile([P, NET, 1], F32, tag="bq")
                nc.sync.dma_start(out=bq_sb, in_=bq_r)
                for tcn in range(NTC):
                    tgt_c = pb.tile([P, NDT, TC], F32R, tag="tgt", bufs=2)
                    nc.sync.dma_start(
                        out=tgt_c, in_=tgt_r[:, :, tcn * TC:(tcn + 1) * TC])
                    for et in range(NET):
                        wq_c = pb.tile([P, NDT, P], F32R, tag="wq", bufs=3)
                        nc.sync.dma_start(
                            out=wq_c, in_=wq_r[:, :, et * P:(et + 1) * P])
                        ps = psB.tile([P, TC], F32, tag="psq", bufs=3)
                        for d in range(NDT):
                            nc.tensor.matmul(
                                ps[:], wq_c[:, d, :], tgt_c[:, d, :],
                                start=(d == 0), stop=(d == NDT - 1))
                        qstage = pb.tile([P, TC], F32R, tag="qst", bufs=4)
                        nc.vector.tensor_scalar_add(
                            qstage[:], ps[:], bq_sb[:, et, :])
                        nc.sync.dma_start(
                            out=qts[:, et, tcn * TC:(tcn + 1) * TC],
                            in_=qstage[:])

            # ---------------- Phase B2: attention + out-proj ----------------
            with tc.tile_pool(name="pc", bufs=1) as pc, \
                 tc.tile_pool(name="psS", bufs=2, space="PSUM") as psS, \
                 tc.tile_pool(name="psM", bufs=2, space="PSUM") as psM, \
                 tc.tile_pool(name="psO", bufs=2, space="PSUM") as psO, \
                 tc.tile_pool(name="psY", bufs=2, space="PSUM") as psY:
                ones = pc.tile([P, P], F32R, tag="ones")
                nc.sync.dma_start(out=ones, in_=ones_in)
                bo_sb = pc.tile([P, NET, 1], F32, tag="bo")
                nc.sync.dma_start(out=bo_sb, in_=bo_r)
                for tcn in range(NTCB):
                    tsl = slice(tcn * TCB, (tcn + 1) * TCB)
                    qt_c = pc.tile([P, NET, TCB], F32R, tag="qt", bufs=2)
                    nc.sync.dma_start(out=qt_c, in_=qts[:, :, tsl])

                    # scores + exp + partial sumexp, per s-tile
                    etiles = []
                    ps_sum = psM.tile([1, TCB], F32, tag="sum", bufs=2)
                    for st in range(NST):
                        ps_s = psS.tile([P, TCB], F32, tag="s", bufs=2)
                        for e in range(NET):
                            nc.tensor.matmul(
                                ps_s[:],
                                KT[:, e, st * P:(st + 1) * P],
                                qt_c[:, e, :],
                                start=(e == 0), stop=(e == NET - 1))
                        etile = pc.tile([P, TCB], F32R, tag=f"e{st}", bufs=1)
                        nc.scalar.activation(
                            etile[:], ps_s[:],
                            mybir.ActivationFunctionType.Exp, scale=float(SCALE))
                        etiles.append(etile)
                        nc.tensor.matmul(
                            ps_sum[:], ones[:, 0:1], etile[:],
                            start=(st == 0), stop=(st == NST - 1))

                    # reciprocal of sumexp, broadcast to 128 partitions
                    recip = pc.tile([1, TCB], F32R, tag="recip", bufs=2)
                    with nc.allow_low_precision(
                            reason="f32r output is bit-identical to f32"):
                        nc.vector.reciprocal(recip[:], ps_sum[:])
                    ps_rbc = psM.tile([P, TCB], F32, tag="sum", bufs=2)
                    nc.tensor.matmul(ps_rbc[:], ones[0:1, :], recip[:],
                                     start=True, stop=True)
                    rbc = pc.tile([P, TCB], F32, tag="rbc", bufs=2)
                    nc.scalar.activation(
                        rbc[:], ps_rbc[:], mybir.ActivationFunctionType.Copy)

                    # OT[e,t] = sum_s V[s,e] * E[s,t]   (unnormalized)
                    ot = pc.tile([P, NET, TCB], F32R, tag="ot", bufs=1)
                    for e in range(NET):
                        ps_o = psO.tile([P, TCB], F32, tag="o", bufs=2)
                        for st in range(NST):
                            nc.tensor.matmul(
                                ps_o[:],
                                V[:, st, e * P:(e + 1) * P],
                                etiles[st][:],
                                start=(st == 0), stop=(st == NST - 1))
                        nc.scalar.activation(
                            ot[:, e, :], ps_o[:],
                            mybir.ActivationFunctionType.Copy)

                    # YT[o,t] = (sum_e woT[e,o] * OT[e,t]) * rbc + bo
                    for ot_i in range(NET):
                        wo_c = pc.tile([P, NET, P], F32R, tag="wo", bufs=3)
                        nc.sync.dma_start(
                            out=wo_c, in_=wo_r[:, :, ot_i * P:(ot_i + 1) * P])
                        ps_y = psY.tile([P, TCB], F32, tag="y", bufs=2)
                        for e in range(NET):
                            nc.tensor.matmul(
                                ps_y[:], wo_c[:, e, :], ot[:, e, :],
                                start=(e == 0), stop=(e == NET - 1))
                        ystage = pc.tile([P, TCB], F32, tag="yst", bufs=4)
                        nc.vector.tensor_mul(ystage[:], ps_y[:], rbc[:])
                        nc.gpsimd.tensor_scalar_add(
                            ystage[:], ystage[:], bo_sb[:, ot_i, :])
                        nc.sync.dma_start(out=y_r[:, ot_i, tsl], in_=ystage[:])

    nc.compile()
    return nc


def prepare_in_maps(target, source, Wq, bq, Wk, bk, Wv, bv, Wo, bo):
    target = np.asarray(target, dtype=np.float32)
    source = np.asarray(source, dtype=np.float32)
    shared = {
        "wqT": np.ascontiguousarray(np.asarray(Wq, np.float32).T),
        "wkT": np.ascontiguousarray(np.asarray(Wk, np.float32).T),
        "wvT": np.ascontiguousarray(np.asarray(Wv, np.float32).T),
        "woT": np.ascontiguousarray(np.asarray(Wo, np.float32).T),
        "bq": np.asarray(bq, np.float32).reshape(D, 1),
        "bk": np.asarray(bk, np.float32).reshape(D, 1),
        "bvb": np.ascontiguousarray(
            np.broadcast_to(np.asarray(bv, np.float32), (P, D))),
        "bo": np.asarray(bo, np.float32).reshape(D, 1),
        "ones_in": np.ones((P, P), np.float32),
    }
    in_maps = []
    for b in range(B):
        m = dict(shared)
        m["tgtT"] = np.ascontiguousarray(target[b].T)
        m["srcT"] = np.ascontiguousarray(source[b].T)
        in_maps.append(m)
    return in_maps


def kernel(target, source, Wq, bq, Wk, bk, Wv, bv, Wo, bo):
    global _CACHED_NC
    if _CACHED_NC is None:
        _CACHED_NC = build_nc()
    nc = _CACHED_NC
    in_maps = prepare_in_maps(target, source, Wq, bq, Wk, bk, Wv, bv, Wo, bo)
    res = run_bass_kernel_spmd(nc, in_maps, list(range(B)))
    out = np.stack([np.ascontiguousarray(res.results[b]["yT"].T)
                    for b in range(B)])
    return out.astype(np.float32)
